# revision 36
# baseline (speedup 1.0000x reference)
"""Trainium2 Bass kernel for nn_AttentionFusion (cross-attention, B=4, LQ=1024,
LKV=4096, D=512, H=4 heads of 128).

Sharding: 8 cores = (batch b in 0..3) x (head-pair hp in 0..1). Core c = 2*b+hp
computes attention for heads {2hp, 2hp+1} of batch b plus its partial
out-projection (tensor-parallel split of Wo). Host sums the partials per batch
plus the bias vector (bo + Wo@bv) — the TP un-shard.

x and enc are transposed + cast to bf16 on the HOST, so the device loads
xT/eT directly (contiguous DMA) and spends zero PE time on input transposes.
(fp8 projections were tried and rejected: independent per-kv fp8 noise on
scores does not average out relative to ctx's own random-sum magnitude, so
final error tracks the ~7% score noise — over the 2e-2 budget.) bk is dropped
(softmax is invariant to a per-query constant); bv/bo are folded into the
host-side bias vector.

v5 structure: heads run SEQUENTIALLY (h0 then h1): PSUM = 3-deep scores
rotation (6 banks) + 1 ctx accumulator (2 banks). All projections are
injected between h0's attention steps; 8 of h1's score/exp tiles are
precomputed ("prefetched") late in the h0 phase into a stash pool so the
ACT-bound h1 phase shortens; h0's finish + out-projection inject into h1's
early steps. ctx matmuls trail their exp by 2 steps (1 for the last steps, to
drain the DVE tree before the tail). GpSimd stays off bulk work (SBUF port
contention halves DVE throughput).

v5 vs v4: (a) xT and eT group-0 load as four 256KB k-chunk DMAs each so the
first projection matmul starts ~5us earlier; (b) the out-projection is
written as TWO partials — h0's (f32) streams to DRAM mid-kernel while the
DMA engines are otherwise idle, h1's (bf16) at the tail — and the host sums
them, removing the final DVE add chain and the cvec broadcast; (c) ctxT
copies split across ACT/DVE; (d) no PE filler.
"""

import numpy as np

B, LQ, LKV, D, H, HD = 4, 1024, 4096, 512, 4, 128
NCORES = 8
SCALE = 1.0 / float(np.sqrt(HD))

_compiled = {}


def _build():
    import concourse.bacc as bacc
    import concourse.mybir as mybir
    from concourse import tile
    from concourse.masks import make_identity

    bf16, f32 = mybir.dt.bfloat16, mybir.dt.float32
    EXP = mybir.ActivationFunctionType.Exp
    IDN = mybir.ActivationFunctionType.Identity

    nc = bacc.Bacc(
        "TRN2",
        target_bir_lowering=False,
        debug=False,
        enable_asserts=True,
        num_devices=NCORES,
    )

    et = nc.dram_tensor("et", [512, LKV], bf16, kind="ExternalInput")
    xt = nc.dram_tensor("xt", [512, LQ], bf16, kind="ExternalInput")
    wqt = nc.dram_tensor("wqt", [128, 1024], bf16, kind="ExternalInput")
    wkt = nc.dram_tensor("wkt", [128, 1024], bf16, kind="ExternalInput")
    wvt = nc.dram_tensor("wvt", [128, 1024], bf16, kind="ExternalInput")
    wot = nc.dram_tensor("wot", [128, 1024], bf16, kind="ExternalInput")
    bq2 = nc.dram_tensor("bq2", [128, 2], f32, kind="ExternalInput")
    out0 = nc.dram_tensor("out0", [LQ, D], f32, kind="ExternalOutput")
    out1 = nc.dram_tensor("out1", [LQ, D], bf16, kind="ExternalOutput")

    with tile.TileContext(nc) as tc:
        with (
            tc.tile_pool(name="const", bufs=1) as const,
            tc.tile_pool(name="big", bufs=1) as big,
            tc.tile_pool(name="expp", bufs=6) as expp,
            tc.tile_pool(name="stash", bufs=10) as stashp,
            tc.tile_pool(name="tree", bufs=11) as treep,
            tc.tile_pool(name="smal", bufs=4) as smal,
            tc.tile_pool(name="nrm0p", bufs=4) as nrm0p,
            tc.tile_pool(name="osb", bufs=4) as osb,
            tc.tile_pool(name="ps", bufs=3, space="PSUM") as psp,
            tc.tile_pool(name="ps_c", bufs=1, space="PSUM") as ps_c,
        ):
            # --- DMAs, single sync ring, in consumption order. xT and eT g0
            # are split into 256KB k-chunks so the first matmuls start as
            # soon as the first chunk lands. ---
            bqsb = const.tile([128, 2], f32)
            nc.sync.dma_start(bqsb[:], bq2[:])
            wq_sb = const.tile([128, 4, 256], bf16)
            nc.sync.dma_start(wq_sb[:], wqt.ap().rearrange("p (k d) -> p k d", k=4))
            xTk = [big.tile([128, LQ], bf16, name=f"xT{k}") for k in range(4)]
            for k in range(2):
                nc.sync.dma_start(xTk[k][:], xt.ap()[128 * k : 128 * (k + 1), :])
            wk_sb = const.tile([128, 4, 256], bf16)
            nc.sync.dma_start(wk_sb[:], wkt.ap().rearrange("p (k d) -> p k d", k=4))
            for k in range(2, 4):
                nc.sync.dma_start(xTk[k][:], xt.ap()[128 * k : 128 * (k + 1), :])
            eT0k = [big.tile([128, 1024], bf16, name=f"eT0_{k}") for k in range(4)]
            for k in range(4):
                nc.sync.dma_start(
                    eT0k[k][:], et.ap()[128 * k : 128 * (k + 1), 0:1024]
                )
            wv_sb = const.tile([128, 4, 256], bf16)
            nc.sync.dma_start(wv_sb[:], wvt.ap().rearrange("p (k d) -> p k d", k=4))
            # g1 then g3 then g2: kT[1][3] is produced early (step 10) so the
            # late-kv prefetches can run mid-phase — its eT group must not
            # arrive last
            eTg = {g: big.tile([128, 4, 1024], bf16, name=f"eT{g}") for g in (1, 2, 3)}
            for g in (1, 3, 2):
                nc.sync.dma_start(
                    eTg[g][:],
                    et.ap()[:, 1024 * g : 1024 * (g + 1)].rearrange(
                        "(k p) q -> p k q", p=128
                    ),
                )
            wo_sb = const.tile([128, 2, D], bf16)
            nc.sync.dma_start(wo_sb[:], wot.ap().rearrange("p (k d) -> p k d", k=2))

            def es(g, k):
                # [128, 1024] view of encoder-transpose chunk (g, k)
                return eT0k[k][:] if g == 0 else eTg[g][:, k, :]

            # --- constants ---
            ones = const.tile([128, 1], f32)
            nc.vector.memset(ones[:], 1.0)
            identb = const.tile([128, 128], bf16)
            make_identity(nc, identb[:])
            # warm the ACT exp table set early (~2.7us table load)
            warm = const.tile([128, 1], f32)
            nc.scalar.activation(warm[:], ones[:], EXP)

            qT = [big.tile([128, LQ], bf16, name=f"qT{h}") for h in range(2)]
            kT = [
                [big.tile([128, 1024], bf16, name=f"kT{h}_{g}") for g in range(4)]
                for h in range(2)
            ]
            v_g = [big.tile([128, 8, 256], bf16, name=f"v{g}") for g in range(4)]

            # --- projection units ---
            def unit_q_both():
                # k-chunk-outer across BOTH head units: 4 matmuls per xT
                # chunk arrival, so the PE chases the chunked DMA instead of
                # idling between chunks
                pss = [
                    psp.tile([128, 1024], f32, name=f"q_ps{t}", tag="sc")
                    for t in range(2)
                ]
                for k in range(4):
                    for t in range(2):
                        for c in range(2):
                            nc.tensor.matmul(
                                pss[t][:, 512 * c : 512 * c + 512],
                                wq_sb[:, k, 128 * t : 128 * t + 128],
                                xTk[k][:, 512 * c : 512 * c + 512],
                                start=(k == 0),
                                stop=(k == 3),
                            )
                for t in range(2):
                    nc.scalar.activation(
                        qT[t][:], pss[t][:], IDN, bias=bqsb[:, t : t + 1]
                    )

            def unit_k(h, g):
                ps = psp.tile([128, 1024], f32, name=f"k_ps{h}{g}", tag="sc")
                for c in range(2):
                    for k in range(4):
                        nc.tensor.matmul(
                            ps[:, 512 * c : 512 * c + 512],
                            wk_sb[:, k, 128 * h : 128 * h + 128],
                            es(g, k)[:, 512 * c : 512 * c + 512],
                            start=(k == 0),
                            stop=(k == 3),
                        )
                # h1's kT copies land in the ACT-slack h0 phase; the (0,0)
                # copy gates the first score wave at startup — ACT is idle
                # then and its PSUM read is faster than DVE's
                if h == 0 and g == 0:
                    nc.scalar.activation(kT[h][g][:], ps[:], IDN)
                elif h == 0:
                    nc.vector.tensor_copy(kT[h][g][:], ps[:])
                else:
                    nc.scalar.activation(kT[h][g][:], ps[:], IDN)

            def unit_v(g, pair):
                ps = psp.tile([128, 1024], f32, name=f"v_ps{g}{pair}", tag="sc")
                for w in range(2):
                    i = 2 * pair + w
                    for k in range(4):
                        nc.tensor.matmul(
                            ps[:, 256 * w : 256 * w + 256],
                            es(g, k)[:, 128 * i : 128 * i + 128],
                            wv_sb[:, k, :],
                            start=(k == 0),
                            stop=(k == 3),
                        )
                nc.vector.tensor_copy(
                    v_g[g][:, 2 * pair : 2 * pair + 2, :],
                    ps[:, 0:512].rearrange("p (w d) -> p w d", w=2),
                )

            # --- attention ---
            ctxT = big.tile([128, 2, LQ], bf16)
            att = {}
            recips = {}
            out0_ap = out0.ap().rearrange("(j p) e -> p j e", p=128)
            out1_ap = out1.ap().rearrange("(j p) e -> p j e", p=128)
            uid = [0]
            ESC = SCALE

            def _tr():
                uid[0] += 1
                return treep.tile([128, LQ], bf16, name=f"tr{uid[0]}", tag="tr")

            def _st(h):
                if h not in att:
                    att[h] = {
                        "ps_ctx": None,
                        "levels": [None] * 6,
                        "pend": [],
                        "pend_late": [],
                        "run": None,
                        "npush": 0,
                        "npop": 0,
                    }
                return att[h]

            def tree_push(h, et_t):
                st = att[h]
                st["npush"] += 1
                if st["run"] is not None:
                    nxt = _tr()
                    nc.vector.tensor_add(nxt[:], st["run"][:], et_t[:])
                    st["run"] = nxt
                    return
                levels = st["levels"]
                cur, lvl = et_t, 0
                while levels[lvl] is not None:
                    nxt = _tr()
                    nc.vector.tensor_add(nxt[:], levels[lvl][:], cur[:])
                    levels[lvl] = None
                    cur, lvl = nxt, lvl + 1
                levels[lvl] = cur
                # collapse into a running sum once only a short tail of
                # pushes remains (h0 sees 32 pushes, h1 only 24 live ones)
                if st["npush"] == (25 if h == 0 else 21):
                    # collapse the tree into a running sum for a short tail
                    run = None
                    for l in range(6):
                        if levels[l] is None:
                            continue
                        if run is None:
                            run = levels[l]
                        else:
                            nxt = _tr()
                            nc.vector.tensor_add(nxt[:], run[:], levels[l][:])
                            run = nxt
                        levels[l] = None
                    st["run"] = run

            def emit_ctx_oldest(h, flush=False, maxpop=2, mindepth=2, late=False):
                # pops the oldest pending (score,exp) tiles into the ctx
                # accumulation. `pend_late` holds prefetched tiles whose exp
                # is long done — they fill pop slots only near/at the end
                # (late=True) so the tail never waits on ACT. Their tree adds
                # are emitted separately (tail_add), decoupled from the pops.
                st = _st(h)
                npop = 0
                while flush or npop < maxpop:
                    if len(st["pend"]) > (0 if flush else mindepth):
                        kt, et_t, g, i, push = st["pend"].pop(0)
                    elif (flush or late) and st["pend_late"]:
                        kt, et_t, g, i, push = st["pend_late"].pop(0)
                    else:
                        break
                    npop += 1
                    st["npop"] += 1
                    last = st["npop"] == 32
                    if st["ps_ctx"] is None:
                        st["ps_ctx"] = ps_c.tile(
                            [128, LQ], f32, name=f"ctx{h}", tag="ctx"
                        )
                    for c in range(2):
                        nc.tensor.matmul(
                            st["ps_ctx"][:, 512 * c : 512 * c + 512],
                            v_g[g][:, i, 128 * h : 128 * h + 128],
                            et_t[:, 512 * c : 512 * c + 512],
                            start=(kt == 0),
                            stop=last,
                        )
                    if push:
                        tree_push(h, et_t)

            def tail_add(h, kt):
                # fold a prefetched tile into the tail running sum; emitted
                # at injection points mid-phase where DVE has slack, NOT at
                # pop time
                st = _st(h)
                et_t = st["stash"][kt]
                if st.get("tailsum") is None:
                    st["tailsum"] = et_t
                else:
                    nxt = _tr()
                    nc.vector.tensor_add(nxt[:], st["tailsum"][:], et_t[:])
                    st["tailsum"] = nxt

            def score_exp(h, kt, pool, late=False):
                st = _st(h)
                g, i = kt // 8, kt % 8
                ps_sc = psp.tile([128, LQ], f32, name=f"sc{h}_{kt}", tag="sc")
                for c in range(2):
                    nc.tensor.matmul(
                        ps_sc[:, 512 * c : 512 * c + 512],
                        kT[h][g][:, 128 * i : 128 * i + 128],
                        qT[h][:, 512 * c : 512 * c + 512],
                        start=True,
                        stop=True,
                    )
                et_t = pool.tile([128, LQ], bf16, name=f"et{h}_{kt}", tag="et")
                nc.scalar.activation(et_t[:], ps_sc[:], EXP, scale=ESC)
                st.setdefault("stash", {})[kt] = et_t
                # late tiles: tree handled by tail_add, not at pop
                (st["pend_late"] if late else st["pend"]).append(
                    (kt, et_t, g, i, not late)
                )

            def attn_step(h, kt, mindepth=2, late=False):
                if kt is not None:
                    score_exp(h, kt, expp)
                emit_ctx_oldest(h, mindepth=mindepth, late=late)

            def finish_a(h):
                st = att[h]
                emit_ctx_oldest(h, flush=True)
                # ctxT gates the out-projection MMs. For h0 (mid-kernel,
                # both engines loaded) split ACT/DVE; for h1 (tail — the
                # prefetched last exps leave ACT idle, while DVE still
                # drains the tree) keep it entirely off DVE.
                if h == 0:
                    nc.scalar.activation(
                        ctxT[:, h, 0:512], st["ps_ctx"][:, 0:512], IDN
                    )
                    nc.vector.tensor_copy(
                        ctxT[:, h, 512:1024], st["ps_ctx"][:, 512:1024]
                    )
                else:
                    # two halves: the first enables outproj mms j0..3 while
                    # the second is still copying
                    for c in range(2):
                        nc.scalar.activation(
                            ctxT[:, h, 512 * c : 512 * c + 512],
                            st["ps_ctx"][:, 512 * c : 512 * c + 512],
                            IDN,
                        )
                # the tree covers every tile: run holds the live tiles,
                # tailsum (h1 only) the prefetched late group, summed
                # mid-phase
                if st.get("tailsum") is None:
                    st["fin"] = st["run"]
                else:
                    fin = _tr()
                    for c in range(2):
                        nc.vector.tensor_add(
                            fin[:, 512 * c : 512 * c + 512],
                            st["run"][:, 512 * c : 512 * c + 512],
                            st["tailsum"][:, 512 * c : 512 * c + 512],
                        )
                    st["fin"] = fin

            def finish_b(h):
                st = att[h]
                fin = st["fin"]
                den = smal.tile([128, 8], f32, name=f"den{h}", tag="den")
                rc = smal.tile([128, 8], f32, name=f"rc{h}", tag="rc")
                # h1's transpose scratch uses the freed ctx bank, keeping all
                # three score-pool buffers for the outproj rotation
                pt = (ps_c if h == 1 else psp).tile(
                    [128, LQ], bf16, name=f"dt{h}", tag=("ctx" if h == 1 else "sc")
                )
                for half in range(2):
                    for j in range(4):
                        jj = 4 * half + j
                        nc.tensor.transpose(
                            pt[:, 128 * jj : 128 * jj + 128],
                            fin[:, 128 * jj : 128 * jj + 128],
                            identb[:],
                        )
                    nc.vector.tensor_reduce(
                        den[:, 4 * half : 4 * half + 4],
                        pt[:, 512 * half : 512 * half + 512].rearrange(
                            "p (j q) -> p j q", j=4
                        ),
                        axis=mybir.AxisListType.X,
                        op=mybir.AluOpType.add,
                    )
                    # per-half reciprocal: scales for j<4 start one reduce
                    # earlier
                    nc.vector.reciprocal(
                        rc[:, 4 * half : 4 * half + 4],
                        den[:, 4 * half : 4 * half + 4],
                    )
                recips[h] = rc

            def outproj0(js):
                # h0 partial: scale on DVE (ACT is the binding engine in the
                # h1 phase these are injected into), stream straight to DRAM
                # through the otherwise-idle mid-kernel DMA window.
                for j in js:
                    p = psp.tile([128, LQ], f32, name=f"o_ps0_{j}", tag="sc")
                    nc.tensor.matmul(
                        p[:, 0:512],
                        ctxT[:, 0, 128 * j : 128 * j + 128],
                        wo_sb[:, 0, :],
                        start=True,
                        stop=True,
                    )
                    n = nrm0p.tile([128, 512], f32, name=f"nrm0_{j}", tag="nrm0")
                    if j % 2 == 0:
                        nc.scalar.activation(
                            n[:], p[:, 0:512], IDN, scale=recips[0][:, j : j + 1]
                        )
                    else:
                        nc.vector.tensor_scalar_mul(
                            n[:], p[:, 0:512], recips[0][:, j : j + 1]
                        )
                    nc.sync.dma_start(out0_ap[:, j, :], n[:])

            def outproj1(js):
                # pack two j-chunks per [128,1024] PSUM tile and borrow the
                # (freed) ctx bank for one pair: 6 matmuls issue back-to-back
                # with no WAR on the scale drain, keeping the PE p-state up
                ps1 = {}
                for pi, pair in enumerate([(0, 1), (2, 3), (4, 5), (6, 7)]):
                    p = psp.tile([128, LQ], f32, name=f"o_ps1_{pi}", tag="sc")
                    for w, j in enumerate(pair):
                        ps1[j] = p[:, 512 * w : 512 * w + 512]
                        nc.tensor.matmul(
                            ps1[j],
                            ctxT[:, 1, 128 * j : 128 * j + 128],
                            wo_sb[:, 1, :],
                            start=True,
                            stop=True,
                        )
                for j in js:
                    n1 = osb.tile([128, 512], bf16, name=f"nrm1_{j}", tag="nrm1")
                    if j % 2 == 0:
                        nc.scalar.activation(
                            n1[:], ps1[j], IDN, scale=recips[1][:, j : j + 1]
                        )
                    else:
                        nc.vector.tensor_scalar_mul(
                            n1[:], ps1[j], recips[1][:, j : j + 1]
                        )
                    nc.sync.dma_start(out1_ap[:, j, :], n1[:])

            # --- schedule ---
            inj = {}

            def add_inj(s, fn):
                inj.setdefault(s, []).append(fn)

            for gi, gn in enumerate((1, 2, 3)):
                base = 8 * gi
                add_inj(base + 0, lambda gn=gn: unit_k(0, gn))
                for pr in range(4):
                    add_inj(base + 1 + pr, lambda gn=gn, pr=pr: unit_v(gn, pr))
            add_inj(5, lambda: unit_k(1, 0))
            add_inj(11, lambda: unit_k(1, 1))
            add_inj(13, lambda: unit_k(1, 2))
            add_inj(10, lambda: unit_k(1, 3))
            # prefetch h1's FIRST two and LAST eight score/exp tiles into the
            # EARLY/MID h0 phase, where the projection-unit injections leave
            # ACT plenty of slack (late-h0 prefetching stalls the score-psum
            # rotation: the 3-deep WAR waits on a lagging exp). The late
            # eight (kt24..31) are the key: they fill the final ctx-pop
            # slots with exp-free work AND their tree adds (tail_add) are
            # emitted mid-phase, so the end of the h1 phase waits on neither
            # ACT nor a serial DVE push chain.
            for p, kt in enumerate((0, 1, 24, 25, 26, 27, 28, 29, 30, 31)):
                s = (7, 9, 11, 13, 15, 17, 19, 21, 23, 25)[p]
                add_inj(s, lambda kt=kt: score_exp(1, kt, stashp, late=(kt >= 24)))
            # fold the late-eight exps into tailsum during h0's LAST steps:
            # the exps are done by step 25 and h0-late DVE only carries one
            # push per step (the h1 phase has no DVE slack left)
            for p, kt in enumerate(range(24, 32)):
                add_inj(24 + p, lambda kt=kt: tail_add(1, kt))
            # finish_a(0) must be emitted BEFORE h1's first ctx matmul: ctx1's
            # PSUM buffer WAR-depends on ctx0's readers (the ctxT copies), and
            # the PE queue is strict FIFO.
            preinj = {32: [lambda: finish_a(0)]}
            # at 35 (not 34): two steps of h1 score/pop matmuls sit between
            # the h0 flush and dt0's transposes in the PE FIFO, covering the
            # DVE latency of fin0
            add_inj(35, lambda: finish_b(0))
            # spread h0's out-projection: its DVE ops per pair otherwise
            # congest the vector queue, delaying h1's tree pushes (et-pool
            # WAR -> exp stall -> PE gap)
            add_inj(44, lambda: outproj0([0, 1]))
            add_inj(46, lambda: outproj0([2, 3]))
            add_inj(48, lambda: outproj0([4, 5]))
            add_inj(50, lambda: outproj0([6, 7]))

            # pre-units: q projections + group-0 k/v
            unit_q_both()
            unit_k(0, 0)
            unit_v(0, 0)
            unit_v(0, 1)
            unit_v(0, 2)
            unit_v(0, 3)

            for s in range(56):
                for fn in preinj.get(s, []):
                    fn()
                if s < 32:
                    # drain h0's pops early too: its flush otherwise stacks
                    # serial tree pushes under the h0 finish at step 32
                    attn_step(0, s, mindepth=(1 if s >= 28 else 2))
                elif s <= 42:
                    # DENSE live scores kt2..23, two per step. ACT sustains 2
                    # exps per 2.6us PE step here, so every exp is done ~12
                    # steps before the phase ends — the tail never waits on
                    # ACT
                    for kt in (2 * (s - 32) + 2, 2 * (s - 32) + 3):
                        attn_step(1, kt, mindepth=2, late=(s >= 40))
                else:
                    # pop-only drain: prefetched late tiles + trailing lives
                    attn_step(
                        1,
                        None,
                        mindepth=(1 if s >= 46 else 2),
                        late=True,
                    )
                for fn in inj.get(s, []):
                    fn()

            finish_a(1)
            finish_b(1)
            outproj1(list(range(8)))

    nc.compile()
    return nc


def _get_nc():
    if "nc" not in _compiled:
        _compiled["nc"] = _build()
    return _compiled["nc"]


def _warr(wt, k, dtype_name="bfloat16", scale=1.0):
    """[k*128, n] -> [128, k*n] so partition p reads one contiguous block."""
    import ml_dtypes

    dt = getattr(ml_dtypes, dtype_name)
    n = wt.shape[1]
    return np.ascontiguousarray(
        (wt * scale).reshape(k, 128, n).transpose(1, 0, 2).reshape(128, k * n)
    ).astype(dt)


def _make_in_maps(x, encoder_feats, Wq, Wk, Wv, bq, bk, bv, Wo, bo):
    import ml_dtypes

    f = np.float32
    bf = ml_dtypes.bfloat16
    x = np.asarray(x, f)
    encoder_feats = np.asarray(encoder_feats, f)
    Wq, Wk, Wv, Wo = (np.asarray(a, f) for a in (Wq, Wk, Wv, Wo))
    bq, bk, bv, bo = (np.asarray(a, f) for a in (bq, bk, bv, bo))

    # host-side transpose + bf16 cast (one copy per batch)
    eT_b = [encoder_feats[b].T.astype(bf) for b in range(B)]  # [512, 4096] bf16
    xT_b = [x[b].T.astype(bf) for b in range(B)]  # [512, 1024] bf16

    # bk is dropped: adding bk to k shifts every score for a given query by the
    # same constant (q . bk), and softmax is invariant to that shift.
    # bv/bo fold into a host-side bias vector added after the gather.
    per_hp = []
    for hp in range(2):
        sl = slice(256 * hp, 256 * hp + 256)
        per_hp.append(
            {
                "wqt": _warr(Wq[sl, :].T, 4),
                "wkt": _warr(Wk[sl, :].T, 4),
                "wvt": _warr(Wv[sl, :].T, 4),
                "wot": _warr(Wo[:, sl].T, 2),
                "bq2": np.ascontiguousarray(bq[sl].reshape(2, 128).T, dtype=f),
            }
        )

    in_maps = []
    for c in range(NCORES):
        b, hp = c // 2, c % 2
        m = {"et": eT_b[b], "xt": xT_b[b]}
        m.update(per_hp[hp])
        in_maps.append(m)
    bias = (bo + Wo @ bv).astype(f)  # [D]
    return in_maps, bias


def kernel(x, encoder_feats, Wq, Wk, Wv, bq, bk, bv, Wo, bo, _trace=False):
    from concourse.bass_utils import run_bass_kernel_spmd

    nc = _get_nc()
    in_maps, bias = _make_in_maps(x, encoder_feats, Wq, Wk, Wv, bq, bk, bv, Wo, bo)
    kw = {}
    if _trace:
        kw = dict(trace=True, trace_cores=[0])
    res = run_bass_kernel_spmd(nc, in_maps, core_ids=list(range(NCORES)), **kw)
    _compiled["last_res"] = res
    out = np.empty((B, LQ, D), np.float32)
    for b in range(B):
        r0, r1 = res.results[2 * b], res.results[2 * b + 1]
        out[b] = (
            r0["out0"]
            + r0["out1"].astype(np.float32)
            + r1["out0"]
            + r1["out1"].astype(np.float32)
            + bias[None, :]
        )
    return out
